# revision 22
# baseline (speedup 1.0000x reference)
"""ArcFace FC loss on 8 TRN2 NeuronCores (classifier/model parallel).

Full inputs in, full (scalar) output out. Classes are sharded 8 ways
(12500/core, zero-padded to 12544 = 98*128). Per core:
  - W and images arrive as fp8e4 (host-cast); per-class inv-norms via
    sum-of-squares split across DVE (STT+accum) and ACT (Square+accum,
    same hw table as Exp), and a DVE bit-trick rsqrt whose chain ops are
    interleaved between the PSUM->fp8 copies so their semaphores are
    posted by execution time (guard 2e-5 keeps padded rows' inv-norms
    fp8-representable).
  - W^T is built on the PE via fp8 DoubleRow matmuls against a
    block-diagonal rhs (two class-groups per instruction) that folds the
    normalization; PSUM -> fp8 wnt casts on DVE.
  - Main GEMM in fp8 DoubleRow (K=256/instruction); exp(64*cos-64) on ACT
    with the image inv-norm in the activation scale and per-row partial
    sums via accum_out. ACT runs only Exp/Square (no table reloads).
  - Target-class cosines from a f32 path (host pre-gathers W[labels],
    pure data movement); one early tpart AllReduce + one final sum-exp
    AllReduce; ArcFace margin + log-sum-exp finish on [128, 8] vectors
    with the log done by DVE exponent/mantissa extraction (no Ln table).
  - Chunk schedule: one small 256-class chunk first, then 8 x 1536.
    DMAs spread over 3 queues; weights prefetched 4 chunks deep; the
    norm pipeline runs 3 chunks ahead so the transposed-weight copies
    are always first in the DVE queue at each chunk boundary.
"""

import os
import sys
from collections import deque

import numpy as np

for _p in ("/opt/trn_rl_repo", "/root/.axon_site/_ro/trn_rl_repo"):
    if os.path.isdir(_p) and _p not in sys.path:
        sys.path.append(_p)

import ml_dtypes

N = 1024
D = 512
C = 100000
NCORES = 8
CSH = C // NCORES          # 12500 classes per core
CPAD = 12544               # 98 * 128
SCALE = 64.0
MARGIN = 0.5
COS_M = float(np.cos(MARGIN))
SIN_M = float(np.sin(MARGIN))
A2 = float((SCALE * SIN_M) ** 2)
M_TILES = N // 128
KG = D // 128

CHUNK_BIG = 1536
CHUNKS = [(0, CPAD - 8 * CHUNK_BIG)] + [
    (CPAD - 8 * CHUNK_BIG + i * CHUNK_BIG, CHUNK_BIG) for i in range(8)
]
N_CHUNKS = len(CHUNKS)     # 9
MAXG = CHUNK_BIG // 128    # 12
ACT_SS_GROUPS = 2          # per big chunk, sum-of-squares groups done on ACT

MAGIC = 0x5F3759DF
LN2 = float(np.log(2.0))
LC0 = 0.0009238244791110461
LC1 = 0.9797604304758613
LC2 = -0.3935457568450573
LC3 = 0.10668815906732054

_CACHE = {}


def _patch_ldw_opt():
    """Enable walrus's ldweights dedup (consecutive matmuls reusing the same
    stationary operand skip the reload)."""
    import concourse.bass_utils as _bu

    if getattr(_bu, "_ldw_patched", False):
        return
    _orig = _bu.run_command

    def _patched(argv, **kw):
        argv = [
            "--enable-ldw-opt=true" if a == "--enable-ldw-opt=false" else a
            for a in argv
        ]
        return _orig(argv, **kw)

    _bu.run_command = _patched
    _bu._ldw_patched = True


def _build():
    import concourse.bass as bass
    import concourse.bacc as bacc
    import concourse.mybir as mybir
    from concourse import tile

    f32 = mybir.dt.float32
    bf16 = mybir.dt.bfloat16
    fp8 = mybir.dt.float8e4
    i32 = mybir.dt.int32
    AF = mybir.ActivationFunctionType
    OP = mybir.AluOpType
    DR = mybir.MatmulPerfMode.DoubleRow

    nc = bacc.Bacc(None, target_bir_lowering=False, debug=False)

    img_ext = nc.declare_dram_parameter("img8", [N, D], fp8, isOutput=False)
    net8_ext = nc.declare_dram_parameter("net8", [D, N], fp8, isOutput=False)
    w_ext = nc.declare_dram_parameter("w", [CPAD, D], fp8, isOutput=False)
    wg_ext = nc.declare_dram_parameter("wg", [N, D], f32, isOutput=False)
    mask_ext = nc.declare_dram_parameter("mask", [128, M_TILES], f32, isOutput=False)
    eye_ext = nc.declare_dram_parameter("eye", [128, 128], f32, isOutput=False)
    out_ext = nc.declare_dram_parameter("out", [1, 1], f32, isOutput=True)

    cc_in_t = nc.dram_tensor("cc_in_t", [128, M_TILES], f32)
    cc_out_t = nc.dram_tensor("cc_out_t", [128, M_TILES], f32, addr_space="Shared")
    cc_in_s = nc.dram_tensor("cc_in_s", [128, M_TILES], f32)
    cc_out_s = nc.dram_tensor("cc_out_s", [128, M_TILES], f32, addr_space="Shared")

    with tile.TileContext(nc) as tc:
        with (
            tc.tile_pool(name="const", bufs=1) as cpool,
            tc.tile_pool(name="wq", bufs=5) as wqpool,
            tc.tile_pool(name="wnt", bufs=3) as wntpool,
            tc.tile_pool(name="nrm", bufs=3) as npool,
            tc.tile_pool(name="diag", bufs=3) as dpool,
            tc.tile_pool(name="et", bufs=3) as epool,
            tc.tile_pool(name="small", bufs=4) as spool,
            tc.tile_pool(name="psumT", bufs=2, space="PSUM") as psumT,
            tc.tile_pool(name="psumM", bufs=2, space="PSUM") as psumM,
        ):
            # ---------------- persistent tiles ----------------
            img_sb = cpool.tile([128, M_TILES, D], fp8)
            wg_sb = cpool.tile([128, M_TILES, D], f32)
            neT = cpool.tile([128, KG, N], fp8)
            mask_sb = cpool.tile([128, M_TILES], f32)
            eye_f = cpool.tile([128, 128], f32)
            eye_bf = cpool.tile([128, 128], bf16)
            eye8 = cpool.tile([128, 128], fp8)
            sums = cpool.tile([128, M_TILES, N_CHUNKS], f32)
            tpart = cpool.tile([128, M_TILES], f32)
            ns2i = cpool.tile([128, M_TILES], f32)
            ri = cpool.tile([128, M_TILES], f32)
            ri64 = cpool.tile([128, M_TILES], f32)
            g2 = cpool.tile([128, M_TILES], f32)
            rgv = cpool.tile([128, M_TILES], f32)
            st_r = cpool.tile([128, M_TILES], f32)
            traw = cpool.tile([128, M_TILES], f32)
            t_sb = cpool.tile([128, M_TILES], f32)
            magic = cpool.tile([128, MAXG], i32)
            bias_m64 = cpool.tile([128, 1], f32)
            ones_sb = cpool.tile([128, 1], f32)
            t_c = cpool.tile([128, M_TILES], f32)
            u_t = cpool.tile([128, M_TILES], f32)
            sin_s = cpool.tile([128, M_TILES], f32)
            m64v = cpool.tile([128, M_TILES], f32)
            e_t = cpool.tile([128, M_TILES], f32)
            e_m = cpool.tile([128, M_TILES], f32)
            smod = cpool.tile([128, M_TILES], f32)
            lgv = cpool.tile([128, M_TILES], f32)
            lv = cpool.tile([128, M_TILES], f32)
            lcol = cpool.tile([128, 1], f32)
            out_sb = cpool.tile([1, 1], f32)

            diag2 = cpool.tile([128, 3, MAXG // 2, 2, 256], fp8)
            nc.vector.memset(magic[:], MAGIC)

            # ---------------- input DMAs ----------------
            nc.gpsimd.dma_start(neT[:], net8_ext[:, :].rearrange("(kg p) n -> p kg n", p=128))
            nc.gpsimd.memset(bias_m64[:], -SCALE)
            nc.gpsimd.memset(ones_sb[:], 1.0)
            nc.gpsimd.memset(diag2[:, 0, 0:1, :, :], 0.0)
            nc.scalar.dma_start(eye_f[:], eye_ext[:, :])
            nc.scalar.dma_start(mask_sb[:], mask_ext[:, :])
            nc.scalar.dma_start(img_sb[:], img_ext[:, :].rearrange("(m p) d -> p m d", p=128))

            nc.vector.tensor_copy(out=eye_bf[:], in_=eye_f[:])
            nc.vector.tensor_copy(out=eye8[:], in_=eye_f[:])
            ps_warm = psumT.tile([128, 2, 256], f32, tag="ps")
            for _w in range(8):
                nc.tensor.matmul(
                    ps_warm[:, _w % 2, 0:128], eye8[:], eye8[:], start=True, stop=True
                )

            # ---------------- DVE op helpers ----------------
            def rsqrt_ops(out_ap, in_ap, ncols, tag, newton=1, guard=True,
                          guard_val=1e-24):
                """List of closures computing out = 1/sqrt(in) elementwise."""
                ti = spool.tile([128, MAXG], i32, tag=tag + "_i")
                uu = spool.tile([128, MAXG], f32, tag=tag + "_u")
                ops = []
                if guard:
                    ops.append(lambda: nc.vector.tensor_scalar_max(
                        out=in_ap, in0=in_ap, scalar1=guard_val))
                ops.append(lambda: nc.vector.tensor_scalar(
                    out=ti[:, :ncols], in0=in_ap.bitcast(i32), scalar1=1,
                    scalar2=None, op0=OP.arith_shift_right))
                ops.append(lambda: nc.vector.tensor_tensor(
                    out=out_ap.bitcast(i32), in0=magic[:, :ncols],
                    in1=ti[:, :ncols], op=OP.subtract))
                for _ in range(newton):
                    ops.append(lambda: nc.vector.tensor_tensor(
                        out=uu[:, :ncols], in0=in_ap, in1=out_ap, op=OP.mult))
                    ops.append(lambda: nc.vector.scalar_tensor_tensor(
                        out=uu[:, :ncols], in0=uu[:, :ncols], scalar=-0.5,
                        in1=out_ap, op0=OP.mult, op1=OP.mult))
                    ops.append(lambda: nc.vector.scalar_tensor_tensor(
                        out=out_ap, in0=uu[:, :ncols], scalar=1.5,
                        in1=out_ap, op0=OP.add, op1=OP.mult))
                return ops

            def fastlog_dve(out_ap, in_ap, ncols, tag):
                ei = spool.tile([128, M_TILES], i32, tag=tag + "_e")
                ef = spool.tile([128, M_TILES], f32, tag=tag + "_f")
                mi = spool.tile([128, M_TILES], i32, tag=tag + "_m")
                mu = spool.tile([128, M_TILES], f32, tag=tag + "_mu")
                pp = spool.tile([128, M_TILES], f32, tag=tag + "_p")
                nc.vector.tensor_scalar(
                    out=ei[:, :ncols], in0=in_ap.bitcast(i32), scalar1=23,
                    scalar2=None, op0=OP.arith_shift_right)
                nc.vector.tensor_copy(out=ef[:, :ncols], in_=ei[:, :ncols])
                nc.vector.tensor_scalar(
                    out=mi[:, :ncols], in0=in_ap.bitcast(i32),
                    scalar1=0x7FFFFF, op0=OP.bitwise_and,
                    scalar2=0x3F800000, op1=OP.bitwise_or)
                nc.vector.tensor_scalar(
                    out=mu[:, :ncols], in0=mi[:, :ncols].bitcast(f32),
                    scalar1=1.0, scalar2=None, op0=OP.subtract)
                nc.vector.tensor_scalar(
                    out=pp[:, :ncols], in0=mu[:, :ncols], scalar1=LC3,
                    op0=OP.mult, scalar2=LC2, op1=OP.add)
                nc.vector.tensor_tensor(
                    out=pp[:, :ncols], in0=pp[:, :ncols], in1=mu[:, :ncols], op=OP.mult)
                nc.vector.tensor_scalar(
                    out=pp[:, :ncols], in0=pp[:, :ncols], scalar1=LC1,
                    scalar2=None, op0=OP.add)
                nc.vector.tensor_tensor(
                    out=pp[:, :ncols], in0=pp[:, :ncols], in1=mu[:, :ncols], op=OP.mult)
                nc.vector.tensor_scalar(
                    out=pp[:, :ncols], in0=pp[:, :ncols], scalar1=LC0,
                    scalar2=None, op0=OP.add)
                nc.vector.scalar_tensor_tensor(
                    out=out_ap, in0=ef[:, :ncols], scalar=LN2, in1=pp[:, :ncols],
                    op0=OP.mult, op1=OP.add)

            # deferred DVE work, drained between transposed-weight copies so
            # chained ops find their semaphores already posted
            dve_fillers = deque()

            def drain_fillers(k):
                for _ in range(min(k, len(dve_fillers))):
                    dve_fillers.popleft()()

            # ---------------- per-chunk stages ----------------
            def prep_dma(cc):
                c0, cn = CHUNKS[cc]
                ng = cn // 128
                wq = wqpool.tile([128, MAXG, D], fp8, tag="wq")
                nc.sync.dma_start(
                    wq[:, :ng, :],
                    w_ext[c0 : c0 + cn, :].rearrange("(g p) d -> p g d", p=128),
                )
                return wq

            def squares_act(cc, wq, ns2):
                """ACT-side sum-of-squares groups (Square is in the Exp table)."""
                ng = CHUNKS[cc][1] // 128
                for g in range(max(0, ng - ACT_SS_GROUPS), ng):
                    sq = npool.tile([128, D], bf16, tag="asq")
                    nc.scalar.activation(
                        out=sq[:], in_=wq[:, g, :], func=AF.Square,
                        accum_out=ns2[:, g : g + 1],
                    )

            def sumsq_ops(cc, wq, ns2):
                ng = CHUNKS[cc][1] // 128
                hi = ng if ng <= ACT_SS_GROUPS else ng - ACT_SS_GROUPS
                ops = []
                for g in range(hi):
                    def _f(g=g):
                        sq = npool.tile([128, D], bf16, tag="wsq", name="sq")
                        nc.vector.scalar_tensor_tensor(
                            out=sq[:], in0=wq[:, g, :], scalar=1.0,
                            in1=wq[:, g, :], op0=OP.mult, op1=OP.mult,
                            accum_out=ns2[:, g : g + 1],
                        )
                    ops.append(_f)
                return ops

            def sumsq_dve(cc, wq, ns2):
                for op in sumsq_ops(cc, wq, ns2):
                    op()

            def emit_interleaved(*op_lists):
                ls = [list(l) for l in op_lists]
                while any(ls):
                    for l in ls:
                        if l:
                            l.pop(0)()

            def alloc_ns2():
                return npool.tile([128, MAXG], f32, tag="ns2", name="ns2")

            def alloc_rwc():
                return npool.tile([128, MAXG], f32, tag="rwc", name="rwc")

            def prep_diag(cc, rwc):
                ng = CHUNKS[cc][1] // 128
                sl = cc % 3
                for j in range(ng // 2):
                    a_bc, b_bc = bass.broadcast_tensor_aps(
                        eye8[:], rwc[:, 2 * j : 2 * j + 1]
                    )
                    nc.gpsimd.tensor_tensor(
                        out=diag2[:, sl, j, 0, 0:128], in0=a_bc, in1=b_bc, op=OP.mult
                    )
                    a_bc, b_bc = bass.broadcast_tensor_aps(
                        eye8[:], rwc[:, 2 * j + 1 : 2 * j + 2]
                    )
                    nc.gpsimd.tensor_tensor(
                        out=diag2[:, sl, j, 1, 128:256], in0=a_bc, in1=b_bc, op=OP.mult
                    )
                return sl

            def t_pe(cc, wq, sl):
                """DoubleRow transpose+normalize: two class-groups per matmul
                via the block-diagonal rhs; 2 PSUM tiles (dg pairs) per pair."""
                ng = CHUNKS[cc][1] // 128
                ps_list = []
                for j in range(ng // 2):
                    for half in range(2):
                        ps = psumT.tile([128, 2, 256], f32, tag="ps", name="ps")
                        for dg2 in range(2):
                            dg = half * 2 + dg2
                            nc.tensor.matmul(
                                ps[:, dg2, :],
                                wq[:, 2 * j : 2 * j + 2, dg * 128 : (dg + 1) * 128],
                                diag2[:, sl, j, :, :],
                                start=True,
                                stop=True,
                                perf_mode=DR,
                            )
                        ps_list.append((j, half, ps))
                return ps_list

            def copies_dve(cc, ps_list):
                """PSUM -> fp8 wnt casts, with filler ops drained in between."""
                wnt = wntpool.tile([128, KG, CHUNK_BIG], fp8, tag="wnt")
                for (j, half, ps) in ps_list:
                    nc.vector.tensor_copy(
                        out=wnt[:, 2 * half : 2 * half + 2,
                                2 * j * 128 : (2 * j + 2) * 128],
                        in_=ps[:],
                    )
                    drain_fillers(2)
                return wnt

            def gmain(cc, wnt):
                cn = CHUNKS[cc][1]
                for m in range(M_TILES):
                    pm = psumM.tile([128, CHUNK_BIG], f32, tag="pm")
                    for kp in range(2):
                        for b0 in range(0, cn, 512):
                            bn = min(512, cn - b0)
                            nc.tensor.matmul(
                                pm[:, b0 : b0 + bn],
                                neT[:, 2 * kp : 2 * kp + 2, m * 128 : (m + 1) * 128],
                                wnt[:, 2 * kp : 2 * kp + 2, b0 : b0 + bn],
                                start=(kp == 0),
                                stop=(kp == 1),
                                perf_mode=DR,
                            )
                    et = epool.tile([128, CHUNK_BIG], bf16, tag="et")
                    nc.scalar.activation(
                        out=et[:, :cn],
                        in_=pm[:, :cn],
                        func=AF.Exp,
                        bias=bias_m64[:],
                        scale=ri64[:, m : m + 1],
                        accum_out=sums[:, m, cc : cc + 1],
                    )

            # ---------------- one-time work (fillers / gpsimd) ----------------
            def emit_image_norms():
                for m in range(M_TILES):
                    sq = spool.tile([128, D], bf16, tag="isq")
                    nc.vector.scalar_tensor_tensor(
                        out=sq[:], in0=img_sb[:, m, :], scalar=1.0,
                        in1=img_sb[:, m, :], op0=OP.mult, op1=OP.mult,
                        accum_out=ns2i[:, m : m + 1],
                    )
                for op in rsqrt_ops(ri[:], ns2i[:], M_TILES, "ri", newton=2):
                    op()
                nc.vector.tensor_scalar_mul(out=ri64[:], in0=ri[:], scalar1=SCALE)

            def target_fillers_a():
                # raw dot products img.wg and |wg|^2 (both f32-exact)
                ops = []
                for m in range(M_TILES):
                    def _fr(m=m):
                        sq = npool.tile([128, D], f32, tag="tsq")
                        nc.vector.scalar_tensor_tensor(
                            out=sq[:], in0=wg_sb[:, m, :], scalar=1.0,
                            in1=img_sb[:, m, :], op0=OP.mult, op1=OP.mult,
                            accum_out=traw[:, m : m + 1],
                        )
                    ops.append(_fr)
                for m in range(M_TILES):
                    def _fg(m=m):
                        sq = npool.tile([128, D], f32, tag="gsq")
                        nc.vector.scalar_tensor_tensor(
                            out=sq[:], in0=wg_sb[:, m, :], scalar=1.0,
                            in1=wg_sb[:, m, :], op0=OP.mult, op1=OP.mult,
                            accum_out=g2[:, m : m + 1],
                        )
                    ops.append(_fg)
                return ops

            def target_fillers_b():
                # tpart = traw * ri * rsqrt(|wg|^2) * mask
                ops = [lambda: nc.vector.tensor_tensor(
                    out=rgv[:], in0=rgv[:], in1=mask_sb[:], op=OP.mult)]
                ops.append(lambda: nc.vector.tensor_tensor(
                    out=tpart[:], in0=traw[:], in1=ri[:], op=OP.mult))
                ops.append(lambda: nc.vector.tensor_tensor(
                    out=tpart[:], in0=tpart[:], in1=rgv[:], op=OP.mult))
                return ops

            def tpath_fillers():
                ops = []
                ops.append(lambda: nc.vector.tensor_scalar(
                    out=t_c[:], in0=t_sb[:], scalar1=-1.0, scalar2=1.0,
                    op0=OP.max, op1=OP.min))
                ops.append(lambda: nc.vector.tensor_tensor(
                    out=u_t[:], in0=t_c[:], in1=t_c[:], op=OP.mult))
                ops.append(lambda: nc.vector.tensor_scalar(
                    out=u_t[:], in0=u_t[:], scalar1=-A2, scalar2=A2,
                    op0=OP.mult, op1=OP.add))
                ops.append(lambda: nc.vector.tensor_scalar_max(
                    out=u_t[:], in0=u_t[:], scalar1=1e-30))
                ops.extend(rsqrt_ops(sin_s[:], u_t[:], M_TILES, "ss",
                                     newton=2, guard=False))
                ops.append(lambda: nc.vector.tensor_tensor(
                    out=sin_s[:], in0=sin_s[:], in1=u_t[:], op=OP.mult))
                ops.append(lambda: nc.vector.scalar_tensor_tensor(
                    out=m64v[:], in0=t_c[:], scalar=SCALE * COS_M, in1=sin_s[:],
                    op0=OP.mult, op1=OP.subtract))
                return ops

            # ---------------- software pipeline ----------------
            wq_t = [None] * N_CHUNKS
            ns_t = [None] * N_CHUNKS
            rw_t = [None] * N_CHUNKS
            dg_t = [None] * N_CHUNKS
            ps_t = [None] * N_CHUNKS
            wnt_t = [None] * N_CHUNKS

            def norms_direct(c, with_act):
                ns_t[c] = alloc_ns2()
                if with_act:
                    squares_act(c, wq_t[c], ns_t[c])
                sumsq_dve(c, wq_t[c], ns_t[c])
                rw_t[c] = alloc_rwc()
                ngc = CHUNKS[c][1] // 128
                for op in rsqrt_ops(rw_t[c][:, :ngc], ns_t[c][:, :ngc], ngc,
                                    "rw", guard_val=2e-5):
                    op()

            for c in range(4):
                wq_t[c] = prep_dma(c)

            # chunk-0 norms + image norms with their rsqrt chains
            # round-robin interleaved (posted semaphores, no 550ns stalls)
            ns_t[0] = alloc_ns2()
            sumsq_dve(0, wq_t[0], ns_t[0])
            for m in range(M_TILES):
                sq = spool.tile([128, D], bf16, tag="isq")
                nc.vector.scalar_tensor_tensor(
                    out=sq[:], in0=img_sb[:, m, :], scalar=1.0,
                    in1=img_sb[:, m, :], op0=OP.mult, op1=OP.mult,
                    accum_out=ns2i[:, m : m + 1],
                )
            rw_t[0] = alloc_rwc()
            ng0 = CHUNKS[0][1] // 128
            emit_interleaved(
                rsqrt_ops(rw_t[0][:, :ng0], ns_t[0][:, :ng0], ng0, "rw",
                          guard_val=2e-5),
                rsqrt_ops(ri[:], ns2i[:], M_TILES, "ri", newton=1),
            )
            nc.vector.tensor_scalar_mul(out=ri64[:], in0=ri[:], scalar1=SCALE)
            dg_t[0] = prep_diag(0, rw_t[0])
            nc.gpsimd.memset(diag2[:, 0, 1:, :, :], 0.0)
            nc.gpsimd.memset(diag2[:, 1:3, :, :, :], 0.0)
            nc.sync.dma_start(wg_sb[:], wg_ext[:, :].rearrange("(m p) d -> p m d", p=128))
            ps_t[0] = t_pe(0, wq_t[0], dg_t[0])
            wnt_t[0] = copies_dve(0, ps_t[0])
            # chunk 1 norms, chain interleaved with chunk-2 sum-of-squares
            ns_t[1] = alloc_ns2()
            squares_act(1, wq_t[1], ns_t[1])
            sumsq_dve(1, wq_t[1], ns_t[1])
            ns_t[2] = alloc_ns2()
            squares_act(2, wq_t[2], ns_t[2])
            rw_t[1] = alloc_rwc()
            ng1 = CHUNKS[1][1] // 128
            emit_interleaved(
                rsqrt_ops(rw_t[1][:, :ng1], ns_t[1][:, :ng1], ng1, "rw",
                          guard_val=2e-5),
                sumsq_ops(2, wq_t[2], ns_t[2]),
            )
            dg_t[1] = prep_diag(1, rw_t[1])
            rw_t[2] = alloc_rwc()
            ng2 = CHUNKS[2][1] // 128
            dve_fillers.extend(
                rsqrt_ops(rw_t[2][:, :ng2], ns_t[2][:, :ng2], ng2, "rw",
                          guard_val=2e-5)
            )

            for cc in range(N_CHUNKS):
                if cc + 4 < N_CHUNKS:
                    wq_t[cc + 4] = prep_dma(cc + 4)
                if cc + 1 < N_CHUNKS:
                    ps_t[cc + 1] = t_pe(cc + 1, wq_t[cc + 1], dg_t[cc + 1])
                    wnt_t[cc + 1] = copies_dve(cc + 1, ps_t[cc + 1])
                if cc + 3 < N_CHUNKS:
                    ns_t[cc + 3] = alloc_ns2()
                    sumsq_dve(cc + 3, wq_t[cc + 3], ns_t[cc + 3])
                    rw_t[cc + 3] = alloc_rwc()
                    ngx = CHUNKS[cc + 3][1] // 128
                    dve_fillers.extend(
                        rsqrt_ops(rw_t[cc + 3][:, :ngx], ns_t[cc + 3][:, :ngx],
                                  ngx, "rw", guard_val=2e-5)
                    )
                if cc + 2 < N_CHUNKS:
                    dg_t[cc + 2] = prep_diag(cc + 2, rw_t[cc + 2])
                if cc == N_CHUNKS - 1:
                    # t-path directly on DVE (may wait on the tpart collective;
                    # nothing downstream on DVE this iteration)
                    drain_fillers(len(dve_fillers))
                    for op in tpath_fillers():
                        op()
                gmain(cc, wnt_t[cc])
                if cc == N_CHUNKS - 1:
                    nc.scalar.activation(
                        out=e_t[:], in_=t_c[:], func=AF.Exp, scale=SCALE,
                        bias=bias_m64[:])
                    nc.scalar.activation(
                        out=e_m[:], in_=m64v[:], func=AF.Exp, scale=1.0,
                        bias=bias_m64[:])
                if cc + 3 < N_CHUNKS:
                    squares_act(cc + 3, wq_t[cc + 3], ns_t[cc + 3])

                if cc == 0:
                    dve_fillers.extend(target_fillers_a())
                elif cc == 1:
                    dve_fillers.extend(
                        rsqrt_ops(rgv[:], g2[:], M_TILES, "rg", newton=2)
                    )
                elif cc == 2:
                    dve_fillers.extend(target_fillers_b())
                elif cc == 3:
                    # tpart AllReduce (the DMA waits on tpart via data deps)
                    nc.sync.dma_start(out=cc_in_t[:, :], in_=tpart[:])
                    nc.gpsimd.collective_compute(
                        "AllReduce", OP.add,
                        replica_groups=[list(range(NCORES))],
                        ins=[cc_in_t[:, :].opt()],
                        outs=[cc_out_t[:, :].opt()],
                    )
                elif cc == 5:
                    nc.sync.dma_start(out=t_sb[:], in_=cc_out_t[:, :])

            # single final sum-exp AllReduce
            stot = spool.tile([128, M_TILES], f32, tag="sa_l")
            nc.vector.tensor_reduce(
                out=stot[:], in_=sums[:], axis=mybir.AxisListType.X, op=OP.add
            )
            nc.sync.dma_start(out=cc_in_s[:, :], in_=stot[:])
            nc.gpsimd.collective_compute(
                "AllReduce", OP.add,
                replica_groups=[list(range(NCORES))],
                ins=[cc_in_s[:, :].opt()],
                outs=[cc_out_s[:, :].opt()],
            )
            nc.sync.dma_start(out=st_r[:], in_=cc_out_s[:, :])

            # ---------------- final loss ----------------
            nc.vector.tensor_tensor(out=smod[:], in0=st_r[:], in1=e_t[:], op=OP.subtract)
            nc.vector.tensor_tensor(out=smod[:], in0=smod[:], in1=e_m[:], op=OP.add)
            fastlog_dve(lgv[:], smod[:], M_TILES, "lg")
            nc.vector.scalar_tensor_tensor(
                out=lv[:], in0=lgv[:], scalar=SCALE - 127.0 * LN2, in1=m64v[:],
                op0=OP.add, op1=OP.subtract,
            )
            nc.vector.tensor_reduce(
                out=lcol[:], in_=lv[:], axis=mybir.AxisListType.X, op=OP.add
            )
            pf = psumT.tile([1, 1], f32, tag="ps")
            nc.tensor.matmul(pf[:], ones_sb[:], lcol[:], start=True, stop=True)
            nc.vector.tensor_scalar_mul(out=out_sb[:], in0=pf[:], scalar1=1.0 / N)
            nc.sync.dma_start(out=out_ext[:, :], in_=out_sb[:])

    nc.compile()
    return nc


def _prep_in_maps(images, labels, weight):
    images = np.ascontiguousarray(np.asarray(images, dtype=np.float32))
    labels = np.asarray(labels).astype(np.int64).reshape(N)
    weight = np.asarray(weight, dtype=np.float32)
    eye = np.eye(128, dtype=np.float32)

    net8 = np.ascontiguousarray(images.T).astype(ml_dtypes.float8_e4m3)
    img8 = images.astype(ml_dtypes.float8_e4m3)
    wg = np.ascontiguousarray(weight[labels])  # [N, D] f32, pure gather

    in_maps = []
    for i in range(NCORES):
        wp = np.zeros((CPAD, D), dtype=ml_dtypes.float8_e4m3)
        wp[:CSH] = weight[i * CSH : (i + 1) * CSH].astype(ml_dtypes.float8_e4m3)
        inside = (labels >= i * CSH) & (labels < (i + 1) * CSH)
        mask = inside.astype(np.float32).reshape(M_TILES, 128).T.copy()
        in_maps.append(
            {
                "img8": img8,
                "net8": net8,
                "w": wp,
                "wg": wg,
                "mask": mask,
                "eye": eye,
            }
        )
    return in_maps


LAST_EXEC_TIME_NS = None
LAST_TRACE = None


def _install_ntff_hook():
    """The agent image's antenv lacks axon_hooks; synthesize it from trn_boot's
    ctypes NTFF driver so run_bass_kernel_spmd(trace=True) can profile."""
    import types

    if "antenv.axon_hooks" in sys.modules:
        return
    try:
        from trn_agent_boot.trn_boot import _ntff_profile_via_ctypes

        hook = _ntff_profile_via_ctypes("/opt/axon/libaxon_pjrt.so")
    except Exception:
        hook = None
    mod = types.ModuleType("antenv.axon_hooks")
    mod._hook = hook
    mod.get_axon_ntff_profile_hook = lambda: mod._hook
    mod.set_axon_ntff_profile_hook = lambda h: setattr(mod, "_hook", h)
    sys.modules["antenv.axon_hooks"] = mod
    import antenv

    antenv.axon_hooks = mod


def kernel(images, labels, weight):
    global LAST_EXEC_TIME_NS, LAST_TRACE
    from concourse.bass_utils import run_bass_kernel_spmd

    if "nc" not in _CACHE:
        _CACHE["nc"] = _build()
    nc = _CACHE["nc"]

    in_maps = _prep_in_maps(images, labels, weight)
    trace = bool(int(os.environ.get("KERNEL_TRACE", "0")))
    if trace:
        _install_ntff_hook()
    res = run_bass_kernel_spmd(nc, in_maps, core_ids=list(range(NCORES)), trace=trace)
    LAST_EXEC_TIME_NS = res.exec_time_ns
    LAST_TRACE = res
    out = np.asarray(res.results[0]["out"], dtype=np.float32).reshape(())
    return out


# revision 23
# speedup vs baseline: 1.0795x; 1.0795x over previous
"""ArcFace FC loss on 8 TRN2 NeuronCores (classifier/model parallel).

Full inputs in, full (scalar) output out. Classes are sharded 8 ways
(12500/core, zero-padded to 12544 = 98*128). Per core:
  - W and images arrive as fp8e4 (host-cast); per-class inv-norms via
    sum-of-squares split across DVE (STT+accum) and ACT (Square+accum,
    same hw table as Exp), and a DVE bit-trick rsqrt whose chain ops are
    interleaved between the PSUM->fp8 copies so their semaphores are
    posted by execution time (guard 2e-5 keeps padded rows' inv-norms
    fp8-representable).
  - W^T is built on the PE via fp8 DoubleRow matmuls against a
    block-diagonal rhs (two class-groups per instruction) that folds the
    normalization; PSUM -> fp8 wnt casts on DVE.
  - Main GEMM in fp8 DoubleRow (K=256/instruction); exp(64*cos-64) on ACT
    with the image inv-norm in the activation scale and per-row partial
    sums via accum_out. ACT runs only Exp/Square (no table reloads).
  - Target-class cosines from a f32 path (host pre-gathers W[labels],
    pure data movement); one early tpart AllReduce + one final sum-exp
    AllReduce; ArcFace margin + log-sum-exp finish on [128, 8] vectors
    with the log done by DVE exponent/mantissa extraction (no Ln table).
  - Chunk schedule: one small 256-class chunk first, then 8 x 1536.
    DMAs spread over 3 queues; weights prefetched 4 chunks deep; the
    norm pipeline runs 3 chunks ahead so the transposed-weight copies
    are always first in the DVE queue at each chunk boundary.
"""

import os
import sys
from collections import deque

import numpy as np

for _p in ("/opt/trn_rl_repo", "/root/.axon_site/_ro/trn_rl_repo"):
    if os.path.isdir(_p) and _p not in sys.path:
        sys.path.append(_p)

import ml_dtypes

N = 1024
D = 512
C = 100000
NCORES = 8
CSH = C // NCORES          # 12500 classes per core
CPAD = 12544               # 98 * 128
SCALE = 64.0
MARGIN = 0.5
COS_M = float(np.cos(MARGIN))
SIN_M = float(np.sin(MARGIN))
A2 = float((SCALE * SIN_M) ** 2)
M_TILES = N // 128
KG = D // 128

CHUNK_BIG = 1536
CHUNKS = [(0, CPAD - 8 * CHUNK_BIG)] + [
    (CPAD - 8 * CHUNK_BIG + i * CHUNK_BIG, CHUNK_BIG) for i in range(8)
]
N_CHUNKS = len(CHUNKS)     # 9
MAXG = CHUNK_BIG // 128    # 12
ACT_SS_GROUPS = 2          # per big chunk, sum-of-squares groups done on ACT

MAGIC = 0x5F3759DF
LN2 = float(np.log(2.0))
LC0 = 0.0009238244791110461
LC1 = 0.9797604304758613
LC2 = -0.3935457568450573
LC3 = 0.10668815906732054

_CACHE = {}


def _patch_ldw_opt():
    """Enable walrus's ldweights dedup (consecutive matmuls reusing the same
    stationary operand skip the reload)."""
    import concourse.bass_utils as _bu

    if getattr(_bu, "_ldw_patched", False):
        return
    _orig = _bu.run_command

    def _patched(argv, **kw):
        argv = [
            "--enable-ldw-opt=true" if a == "--enable-ldw-opt=false" else a
            for a in argv
        ]
        return _orig(argv, **kw)

    _bu.run_command = _patched
    _bu._ldw_patched = True


def _build():
    import concourse.bass as bass
    import concourse.bacc as bacc
    import concourse.mybir as mybir
    from concourse import tile

    f32 = mybir.dt.float32
    bf16 = mybir.dt.bfloat16
    fp8 = mybir.dt.float8e4
    i32 = mybir.dt.int32
    AF = mybir.ActivationFunctionType
    OP = mybir.AluOpType
    DR = mybir.MatmulPerfMode.DoubleRow

    nc = bacc.Bacc(None, target_bir_lowering=False, debug=False)

    img_ext = nc.declare_dram_parameter("img8", [N, D], fp8, isOutput=False)
    net8_ext = nc.declare_dram_parameter("net8", [D, N], fp8, isOutput=False)
    w_ext = nc.declare_dram_parameter("w", [CPAD, D], fp8, isOutput=False)
    wg_ext = nc.declare_dram_parameter("wg", [N, D], f32, isOutput=False)
    mask_ext = nc.declare_dram_parameter("mask", [128, M_TILES], f32, isOutput=False)
    eye_ext = nc.declare_dram_parameter("eye", [128, 128], f32, isOutput=False)
    out_ext = nc.declare_dram_parameter("out", [1, 1], f32, isOutput=True)

    cc_in_t = nc.dram_tensor("cc_in_t", [128, M_TILES], f32)
    cc_out_t = nc.dram_tensor("cc_out_t", [128, M_TILES], f32, addr_space="Shared")
    cc_in_s = nc.dram_tensor("cc_in_s", [128, M_TILES], f32)
    cc_out_s = nc.dram_tensor("cc_out_s", [128, M_TILES], f32, addr_space="Shared")

    with tile.TileContext(nc) as tc:
        with (
            tc.tile_pool(name="const", bufs=1) as cpool,
            tc.tile_pool(name="wq", bufs=5) as wqpool,
            tc.tile_pool(name="wnt", bufs=3) as wntpool,
            tc.tile_pool(name="nrm", bufs=3) as npool,
            tc.tile_pool(name="diag", bufs=3) as dpool,
            tc.tile_pool(name="et", bufs=3) as epool,
            tc.tile_pool(name="small", bufs=4) as spool,
            tc.tile_pool(name="psumT", bufs=2, space="PSUM") as psumT,
            tc.tile_pool(name="psumM", bufs=2, space="PSUM") as psumM,
        ):
            # ---------------- persistent tiles ----------------
            img_sb = cpool.tile([128, M_TILES, D], fp8)
            wg_sb = cpool.tile([128, M_TILES, D], f32)
            neT = cpool.tile([128, KG, N], fp8)
            mask_sb = cpool.tile([128, M_TILES], f32)
            eye_f = cpool.tile([128, 128], f32)
            eye_bf = cpool.tile([128, 128], bf16)
            eye8 = cpool.tile([128, 128], fp8)
            sums = cpool.tile([128, M_TILES, N_CHUNKS], f32)
            tpart = cpool.tile([128, M_TILES], f32)
            ns2i = cpool.tile([128, M_TILES], f32)
            ri = cpool.tile([128, M_TILES], f32)
            ri64 = cpool.tile([128, M_TILES], f32)
            g2 = cpool.tile([128, M_TILES], f32)
            rgv = cpool.tile([128, M_TILES], f32)
            st_r = cpool.tile([128, M_TILES], f32)
            traw = cpool.tile([128, M_TILES], f32)
            t_sb = cpool.tile([128, M_TILES], f32)
            magic = cpool.tile([128, MAXG], i32)
            bias_m64 = cpool.tile([128, 1], f32)
            ones_sb = cpool.tile([128, 1], f32)
            t_c = cpool.tile([128, M_TILES], f32)
            u_t = cpool.tile([128, M_TILES], f32)
            sin_s = cpool.tile([128, M_TILES], f32)
            m64v = cpool.tile([128, M_TILES], f32)
            e_t = cpool.tile([128, M_TILES], f32)
            e_m = cpool.tile([128, M_TILES], f32)
            smod = cpool.tile([128, M_TILES], f32)
            lgv = cpool.tile([128, M_TILES], f32)
            lv = cpool.tile([128, M_TILES], f32)
            lcol = cpool.tile([128, 1], f32)
            out_sb = cpool.tile([1, 1], f32)

            diag2 = cpool.tile([128, 3, MAXG // 2, 2, 256], fp8)
            nc.vector.memset(magic[:], MAGIC)

            # ---------------- input DMAs ----------------
            nc.gpsimd.dma_start(neT[:], net8_ext[:, :].rearrange("(kg p) n -> p kg n", p=128))
            nc.gpsimd.memset(bias_m64[:], -SCALE)
            nc.gpsimd.memset(ones_sb[:], 1.0)
            nc.gpsimd.memset(diag2[:, 0, 0:1, :, :], 0.0)
            nc.scalar.dma_start(eye_f[:], eye_ext[:, :])
            nc.scalar.dma_start(mask_sb[:], mask_ext[:, :])
            nc.scalar.dma_start(img_sb[:], img_ext[:, :].rearrange("(m p) d -> p m d", p=128))

            nc.vector.tensor_copy(out=eye_bf[:], in_=eye_f[:])
            nc.vector.tensor_copy(out=eye8[:], in_=eye_f[:])
            ps_warm = psumT.tile([128, 2, 256], f32, tag="ps")
            for _w in range(8):
                nc.tensor.matmul(
                    ps_warm[:, _w % 2, 0:128], eye8[:], eye8[:], start=True, stop=True
                )

            # ---------------- DVE op helpers ----------------
            def rsqrt_ops(out_ap, in_ap, ncols, tag, newton=1, guard=True,
                          guard_val=1e-24):
                """List of closures computing out = 1/sqrt(in) elementwise."""
                ti = spool.tile([128, MAXG], i32, tag=tag + "_i")
                uu = spool.tile([128, MAXG], f32, tag=tag + "_u")
                ops = []
                if guard:
                    ops.append(lambda: nc.vector.tensor_scalar_max(
                        out=in_ap, in0=in_ap, scalar1=guard_val))
                ops.append(lambda: nc.vector.tensor_scalar(
                    out=ti[:, :ncols], in0=in_ap.bitcast(i32), scalar1=1,
                    scalar2=None, op0=OP.arith_shift_right))
                ops.append(lambda: nc.vector.tensor_tensor(
                    out=out_ap.bitcast(i32), in0=magic[:, :ncols],
                    in1=ti[:, :ncols], op=OP.subtract))
                for _ in range(newton):
                    ops.append(lambda: nc.vector.tensor_tensor(
                        out=uu[:, :ncols], in0=in_ap, in1=out_ap, op=OP.mult))
                    ops.append(lambda: nc.vector.scalar_tensor_tensor(
                        out=uu[:, :ncols], in0=uu[:, :ncols], scalar=-0.5,
                        in1=out_ap, op0=OP.mult, op1=OP.mult))
                    ops.append(lambda: nc.vector.scalar_tensor_tensor(
                        out=out_ap, in0=uu[:, :ncols], scalar=1.5,
                        in1=out_ap, op0=OP.add, op1=OP.mult))
                return ops

            def fastlog_dve(out_ap, in_ap, ncols, tag):
                ei = spool.tile([128, M_TILES], i32, tag=tag + "_e")
                ef = spool.tile([128, M_TILES], f32, tag=tag + "_f")
                mi = spool.tile([128, M_TILES], i32, tag=tag + "_m")
                mu = spool.tile([128, M_TILES], f32, tag=tag + "_mu")
                pp = spool.tile([128, M_TILES], f32, tag=tag + "_p")
                nc.vector.tensor_scalar(
                    out=ei[:, :ncols], in0=in_ap.bitcast(i32), scalar1=23,
                    scalar2=None, op0=OP.arith_shift_right)
                nc.vector.tensor_copy(out=ef[:, :ncols], in_=ei[:, :ncols])
                nc.vector.tensor_scalar(
                    out=mi[:, :ncols], in0=in_ap.bitcast(i32),
                    scalar1=0x7FFFFF, op0=OP.bitwise_and,
                    scalar2=0x3F800000, op1=OP.bitwise_or)
                nc.vector.tensor_scalar(
                    out=mu[:, :ncols], in0=mi[:, :ncols].bitcast(f32),
                    scalar1=1.0, scalar2=None, op0=OP.subtract)
                nc.vector.tensor_scalar(
                    out=pp[:, :ncols], in0=mu[:, :ncols], scalar1=LC3,
                    op0=OP.mult, scalar2=LC2, op1=OP.add)
                nc.vector.tensor_tensor(
                    out=pp[:, :ncols], in0=pp[:, :ncols], in1=mu[:, :ncols], op=OP.mult)
                nc.vector.tensor_scalar(
                    out=pp[:, :ncols], in0=pp[:, :ncols], scalar1=LC1,
                    scalar2=None, op0=OP.add)
                nc.vector.tensor_tensor(
                    out=pp[:, :ncols], in0=pp[:, :ncols], in1=mu[:, :ncols], op=OP.mult)
                nc.vector.tensor_scalar(
                    out=pp[:, :ncols], in0=pp[:, :ncols], scalar1=LC0,
                    scalar2=None, op0=OP.add)
                nc.vector.scalar_tensor_tensor(
                    out=out_ap, in0=ef[:, :ncols], scalar=LN2, in1=pp[:, :ncols],
                    op0=OP.mult, op1=OP.add)

            # deferred DVE work, drained between transposed-weight copies so
            # chained ops find their semaphores already posted
            dve_fillers = deque()

            def drain_fillers(k):
                for _ in range(min(k, len(dve_fillers))):
                    dve_fillers.popleft()()

            # ---------------- per-chunk stages ----------------
            def prep_dma(cc):
                c0, cn = CHUNKS[cc]
                ng = cn // 128
                wq = wqpool.tile([128, MAXG, D], fp8, tag="wq")
                nc.sync.dma_start(
                    wq[:, :ng, :],
                    w_ext[c0 : c0 + cn, :].rearrange("(g p) d -> p g d", p=128),
                )
                return wq

            def squares_act(cc, wq, ns2):
                """ACT-side sum-of-squares groups (Square is in the Exp table)."""
                ng = CHUNKS[cc][1] // 128
                for g in range(max(0, ng - ACT_SS_GROUPS), ng):
                    sq = npool.tile([128, D], bf16, tag="asq")
                    nc.scalar.activation(
                        out=sq[:], in_=wq[:, g, :], func=AF.Square,
                        accum_out=ns2[:, g : g + 1],
                    )

            def sumsq_ops(cc, wq, ns2):
                ng = CHUNKS[cc][1] // 128
                hi = ng if ng <= ACT_SS_GROUPS else ng - ACT_SS_GROUPS
                ops = []
                for g in range(hi):
                    def _f(g=g):
                        sq = npool.tile([128, D], bf16, tag="wsq", name="sq")
                        nc.vector.scalar_tensor_tensor(
                            out=sq[:], in0=wq[:, g, :], scalar=1.0,
                            in1=wq[:, g, :], op0=OP.mult, op1=OP.mult,
                            accum_out=ns2[:, g : g + 1],
                        )
                    ops.append(_f)
                return ops

            def sumsq_dve(cc, wq, ns2):
                for op in sumsq_ops(cc, wq, ns2):
                    op()

            def emit_interleaved(*op_lists):
                ls = [list(l) for l in op_lists]
                while any(ls):
                    for l in ls:
                        if l:
                            l.pop(0)()

            def alloc_ns2():
                return npool.tile([128, MAXG], f32, tag="ns2", name="ns2")

            def alloc_rwc():
                return npool.tile([128, MAXG], f32, tag="rwc", name="rwc")

            def prep_diag(cc, rwc):
                ng = CHUNKS[cc][1] // 128
                sl = cc % 3
                for j in range(ng // 2):
                    a_bc, b_bc = bass.broadcast_tensor_aps(
                        eye8[:], rwc[:, 2 * j : 2 * j + 1]
                    )
                    nc.gpsimd.tensor_tensor(
                        out=diag2[:, sl, j, 0, 0:128], in0=a_bc, in1=b_bc, op=OP.mult
                    )
                    a_bc, b_bc = bass.broadcast_tensor_aps(
                        eye8[:], rwc[:, 2 * j + 1 : 2 * j + 2]
                    )
                    nc.gpsimd.tensor_tensor(
                        out=diag2[:, sl, j, 1, 128:256], in0=a_bc, in1=b_bc, op=OP.mult
                    )
                return sl

            def t_pe(cc, wq, sl):
                """DoubleRow transpose+normalize: two class-groups per matmul
                via the block-diagonal rhs; 2 PSUM tiles (dg pairs) per pair."""
                ng = CHUNKS[cc][1] // 128
                ps_list = []
                for j in range(ng // 2):
                    for half in range(2):
                        ps = psumT.tile([128, 2, 256], f32, tag="ps", name="ps")
                        for dg2 in range(2):
                            dg = half * 2 + dg2
                            nc.tensor.matmul(
                                ps[:, dg2, :],
                                wq[:, 2 * j : 2 * j + 2, dg * 128 : (dg + 1) * 128],
                                diag2[:, sl, j, :, :],
                                start=True,
                                stop=True,
                                perf_mode=DR,
                            )
                        ps_list.append((j, half, ps))
                return ps_list

            def copies_dve(cc, ps_list):
                """PSUM -> fp8 wnt casts, with filler ops drained in between."""
                wnt = wntpool.tile([128, KG, CHUNK_BIG], fp8, tag="wnt")
                for (j, half, ps) in ps_list:
                    nc.vector.tensor_copy(
                        out=wnt[:, 2 * half : 2 * half + 2,
                                2 * j * 128 : (2 * j + 2) * 128],
                        in_=ps[:],
                    )
                    drain_fillers(2)
                return wnt

            def gmain(cc, wnt):
                cn = CHUNKS[cc][1]
                for m in range(M_TILES):
                    pm = psumM.tile([128, CHUNK_BIG], f32, tag="pm")
                    for kp in range(2):
                        for b0 in range(0, cn, 512):
                            bn = min(512, cn - b0)
                            nc.tensor.matmul(
                                pm[:, b0 : b0 + bn],
                                neT[:, 2 * kp : 2 * kp + 2, m * 128 : (m + 1) * 128],
                                wnt[:, 2 * kp : 2 * kp + 2, b0 : b0 + bn],
                                start=(kp == 0),
                                stop=(kp == 1),
                                perf_mode=DR,
                            )
                    et = epool.tile([128, CHUNK_BIG], bf16, tag="et")
                    nc.scalar.activation(
                        out=et[:, :cn],
                        in_=pm[:, :cn],
                        func=AF.Exp,
                        bias=bias_m64[:],
                        scale=ri64[:, m : m + 1],
                        accum_out=sums[:, m, cc : cc + 1],
                    )

            # ---------------- one-time work (fillers / gpsimd) ----------------
            def emit_image_norms():
                for m in range(M_TILES):
                    sq = spool.tile([128, D], bf16, tag="isq")
                    nc.vector.scalar_tensor_tensor(
                        out=sq[:], in0=img_sb[:, m, :], scalar=1.0,
                        in1=img_sb[:, m, :], op0=OP.mult, op1=OP.mult,
                        accum_out=ns2i[:, m : m + 1],
                    )
                for op in rsqrt_ops(ri[:], ns2i[:], M_TILES, "ri", newton=2):
                    op()
                nc.vector.tensor_scalar_mul(out=ri64[:], in0=ri[:], scalar1=SCALE)

            def target_fillers_a():
                # raw dot products img.wg and |wg|^2 (both f32-exact)
                ops = []
                for m in range(M_TILES):
                    def _fr(m=m):
                        sq = npool.tile([128, D], f32, tag="tsq")
                        nc.vector.scalar_tensor_tensor(
                            out=sq[:], in0=wg_sb[:, m, :], scalar=1.0,
                            in1=img_sb[:, m, :], op0=OP.mult, op1=OP.mult,
                            accum_out=traw[:, m : m + 1],
                        )
                    ops.append(_fr)
                for m in range(M_TILES):
                    def _fg(m=m):
                        sq = npool.tile([128, D], f32, tag="gsq")
                        nc.vector.scalar_tensor_tensor(
                            out=sq[:], in0=wg_sb[:, m, :], scalar=1.0,
                            in1=wg_sb[:, m, :], op0=OP.mult, op1=OP.mult,
                            accum_out=g2[:, m : m + 1],
                        )
                    ops.append(_fg)
                return ops

            def target_fillers_b():
                # tpart = traw * ri * rsqrt(|wg|^2) * mask
                ops = [lambda: nc.vector.tensor_tensor(
                    out=rgv[:], in0=rgv[:], in1=mask_sb[:], op=OP.mult)]
                ops.append(lambda: nc.vector.tensor_tensor(
                    out=tpart[:], in0=traw[:], in1=ri[:], op=OP.mult))
                ops.append(lambda: nc.vector.tensor_tensor(
                    out=tpart[:], in0=tpart[:], in1=rgv[:], op=OP.mult))
                return ops

            def tpath_fillers():
                ops = []
                ops.append(lambda: nc.vector.tensor_scalar(
                    out=t_c[:], in0=t_sb[:], scalar1=-1.0, scalar2=1.0,
                    op0=OP.max, op1=OP.min))
                ops.append(lambda: nc.vector.tensor_tensor(
                    out=u_t[:], in0=t_c[:], in1=t_c[:], op=OP.mult))
                ops.append(lambda: nc.vector.tensor_scalar(
                    out=u_t[:], in0=u_t[:], scalar1=-A2, scalar2=A2,
                    op0=OP.mult, op1=OP.add))
                ops.append(lambda: nc.vector.tensor_scalar_max(
                    out=u_t[:], in0=u_t[:], scalar1=1e-30))
                ops.extend(rsqrt_ops(sin_s[:], u_t[:], M_TILES, "ss",
                                     newton=2, guard=False))
                ops.append(lambda: nc.vector.tensor_tensor(
                    out=sin_s[:], in0=sin_s[:], in1=u_t[:], op=OP.mult))
                ops.append(lambda: nc.vector.scalar_tensor_tensor(
                    out=m64v[:], in0=t_c[:], scalar=SCALE * COS_M, in1=sin_s[:],
                    op0=OP.mult, op1=OP.subtract))
                return ops

            # ---------------- software pipeline ----------------
            wq_t = [None] * N_CHUNKS
            ns_t = [None] * N_CHUNKS
            rw_t = [None] * N_CHUNKS
            dg_t = [None] * N_CHUNKS
            ps_t = [None] * N_CHUNKS
            wnt_t = [None] * N_CHUNKS

            def norms_direct(c, with_act):
                ns_t[c] = alloc_ns2()
                if with_act:
                    squares_act(c, wq_t[c], ns_t[c])
                sumsq_dve(c, wq_t[c], ns_t[c])
                rw_t[c] = alloc_rwc()
                ngc = CHUNKS[c][1] // 128
                for op in rsqrt_ops(rw_t[c][:, :ngc], ns_t[c][:, :ngc], ngc,
                                    "rw", guard_val=2e-5):
                    op()

            for c in range(4):
                wq_t[c] = prep_dma(c)

            # chunk-0 norms + image norms with their rsqrt chains
            # round-robin interleaved (posted semaphores, no 550ns stalls)
            ns_t[0] = alloc_ns2()
            sumsq_dve(0, wq_t[0], ns_t[0])
            for m in range(M_TILES):
                sq = spool.tile([128, D], bf16, tag="isq")
                nc.vector.scalar_tensor_tensor(
                    out=sq[:], in0=img_sb[:, m, :], scalar=1.0,
                    in1=img_sb[:, m, :], op0=OP.mult, op1=OP.mult,
                    accum_out=ns2i[:, m : m + 1],
                )
            rw_t[0] = alloc_rwc()
            ng0 = CHUNKS[0][1] // 128
            emit_interleaved(
                rsqrt_ops(rw_t[0][:, :ng0], ns_t[0][:, :ng0], ng0, "rw",
                          guard_val=2e-5),
                rsqrt_ops(ri[:], ns2i[:], M_TILES, "ri", newton=1),
            )
            nc.vector.tensor_scalar_mul(out=ri64[:], in0=ri[:], scalar1=SCALE)
            dg_t[0] = prep_diag(0, rw_t[0])
            nc.gpsimd.memset(diag2[:, 0, 1:, :, :], 0.0)
            nc.gpsimd.memset(diag2[:, 1:3, :, :, :], 0.0)
            nc.sync.dma_start(wg_sb[:], wg_ext[:, :].rearrange("(m p) d -> p m d", p=128))
            ps_t[0] = t_pe(0, wq_t[0], dg_t[0])
            wnt_t[0] = copies_dve(0, ps_t[0])
            # chunk 1 norms, chain interleaved with chunk-2 sum-of-squares
            ns_t[1] = alloc_ns2()
            squares_act(1, wq_t[1], ns_t[1])
            sumsq_dve(1, wq_t[1], ns_t[1])
            ns_t[2] = alloc_ns2()
            squares_act(2, wq_t[2], ns_t[2])
            rw_t[1] = alloc_rwc()
            ng1 = CHUNKS[1][1] // 128
            emit_interleaved(
                rsqrt_ops(rw_t[1][:, :ng1], ns_t[1][:, :ng1], ng1, "rw",
                          guard_val=2e-5),
                sumsq_ops(2, wq_t[2], ns_t[2]),
            )
            dg_t[1] = prep_diag(1, rw_t[1])
            rw_t[2] = alloc_rwc()
            ng2 = CHUNKS[2][1] // 128
            dve_fillers.extend(
                rsqrt_ops(rw_t[2][:, :ng2], ns_t[2][:, :ng2], ng2, "rw",
                          guard_val=2e-5)
            )

            ta_ops = []
            for cc in range(N_CHUNKS):
                if cc + 4 < N_CHUNKS:
                    wq_t[cc + 4] = prep_dma(cc + 4)
                if cc + 1 < N_CHUNKS:
                    ps_t[cc + 1] = t_pe(cc + 1, wq_t[cc + 1], dg_t[cc + 1])
                    wnt_t[cc + 1] = copies_dve(cc + 1, ps_t[cc + 1])
                if cc + 3 < N_CHUNKS:
                    ns_t[cc + 3] = alloc_ns2()
                    sumsq_dve(cc + 3, wq_t[cc + 3], ns_t[cc + 3])
                    rw_t[cc + 3] = alloc_rwc()
                    ngx = CHUNKS[cc + 3][1] // 128
                    dve_fillers.extend(
                        rsqrt_ops(rw_t[cc + 3][:, :ngx], ns_t[cc + 3][:, :ngx],
                                  ngx, "rw", guard_val=2e-5)
                    )
                if cc + 2 < N_CHUNKS:
                    dg_t[cc + 2] = prep_diag(cc + 2, rw_t[cc + 2])
                if cc == N_CHUNKS - 1:
                    # t-path directly on DVE (may wait on the tpart collective;
                    # nothing downstream on DVE this iteration)
                    drain_fillers(len(dve_fillers))
                    for op in tpath_fillers():
                        op()
                gmain(cc, wnt_t[cc])
                if cc == N_CHUNKS - 1:
                    nc.scalar.activation(
                        out=e_t[:], in_=t_c[:], func=AF.Exp, scale=SCALE,
                        bias=bias_m64[:])
                    nc.scalar.activation(
                        out=e_m[:], in_=m64v[:], func=AF.Exp, scale=1.0,
                        bias=bias_m64[:])
                if cc + 3 < N_CHUNKS:
                    squares_act(cc + 3, wq_t[cc + 3], ns_t[cc + 3])

                if cc == 0:
                    ta_ops[:] = target_fillers_a()
                    dve_fillers.extend(ta_ops[:8])
                elif cc == 1:
                    dve_fillers.extend(ta_ops[8:])
                elif cc == 2:
                    dve_fillers.extend(
                        rsqrt_ops(rgv[:], g2[:], M_TILES, "rg", newton=2)
                    )
                elif cc == 3:
                    dve_fillers.extend(target_fillers_b())
                elif cc == 4:
                    # tpart AllReduce (the DMA waits on tpart via data deps)
                    nc.sync.dma_start(out=cc_in_t[:, :], in_=tpart[:])
                    nc.gpsimd.collective_compute(
                        "AllReduce", OP.add,
                        replica_groups=[list(range(NCORES))],
                        ins=[cc_in_t[:, :].opt()],
                        outs=[cc_out_t[:, :].opt()],
                    )
                elif cc == 6:
                    nc.sync.dma_start(out=t_sb[:], in_=cc_out_t[:, :])

            # single final sum-exp AllReduce
            stot = spool.tile([128, M_TILES], f32, tag="sa_l")
            nc.vector.tensor_reduce(
                out=stot[:], in_=sums[:], axis=mybir.AxisListType.X, op=OP.add
            )
            nc.sync.dma_start(out=cc_in_s[:, :], in_=stot[:])
            nc.gpsimd.collective_compute(
                "AllReduce", OP.add,
                replica_groups=[list(range(NCORES))],
                ins=[cc_in_s[:, :].opt()],
                outs=[cc_out_s[:, :].opt()],
            )
            nc.sync.dma_start(out=st_r[:], in_=cc_out_s[:, :])

            # ---------------- final loss ----------------
            nc.vector.tensor_tensor(out=smod[:], in0=st_r[:], in1=e_t[:], op=OP.subtract)
            nc.vector.tensor_tensor(out=smod[:], in0=smod[:], in1=e_m[:], op=OP.add)
            fastlog_dve(lgv[:], smod[:], M_TILES, "lg")
            nc.vector.scalar_tensor_tensor(
                out=lv[:], in0=lgv[:], scalar=SCALE - 127.0 * LN2, in1=m64v[:],
                op0=OP.add, op1=OP.subtract,
            )
            nc.vector.tensor_reduce(
                out=lcol[:], in_=lv[:], axis=mybir.AxisListType.X, op=OP.add
            )
            pf = psumT.tile([1, 1], f32, tag="ps")
            nc.tensor.matmul(pf[:], ones_sb[:], lcol[:], start=True, stop=True)
            nc.vector.tensor_scalar_mul(out=out_sb[:], in0=pf[:], scalar1=1.0 / N)
            nc.sync.dma_start(out=out_ext[:, :], in_=out_sb[:])

    nc.compile()
    return nc


def _prep_in_maps(images, labels, weight):
    images = np.ascontiguousarray(np.asarray(images, dtype=np.float32))
    labels = np.asarray(labels).astype(np.int64).reshape(N)
    weight = np.asarray(weight, dtype=np.float32)
    eye = np.eye(128, dtype=np.float32)

    net8 = np.ascontiguousarray(images.T).astype(ml_dtypes.float8_e4m3)
    img8 = images.astype(ml_dtypes.float8_e4m3)
    wg = np.ascontiguousarray(weight[labels])  # [N, D] f32, pure gather

    in_maps = []
    for i in range(NCORES):
        wp = np.zeros((CPAD, D), dtype=ml_dtypes.float8_e4m3)
        wp[:CSH] = weight[i * CSH : (i + 1) * CSH].astype(ml_dtypes.float8_e4m3)
        inside = (labels >= i * CSH) & (labels < (i + 1) * CSH)
        mask = inside.astype(np.float32).reshape(M_TILES, 128).T.copy()
        in_maps.append(
            {
                "img8": img8,
                "net8": net8,
                "w": wp,
                "wg": wg,
                "mask": mask,
                "eye": eye,
            }
        )
    return in_maps


LAST_EXEC_TIME_NS = None
LAST_TRACE = None


def _install_ntff_hook():
    """The agent image's antenv lacks axon_hooks; synthesize it from trn_boot's
    ctypes NTFF driver so run_bass_kernel_spmd(trace=True) can profile."""
    import types

    if "antenv.axon_hooks" in sys.modules:
        return
    try:
        from trn_agent_boot.trn_boot import _ntff_profile_via_ctypes

        hook = _ntff_profile_via_ctypes("/opt/axon/libaxon_pjrt.so")
    except Exception:
        hook = None
    mod = types.ModuleType("antenv.axon_hooks")
    mod._hook = hook
    mod.get_axon_ntff_profile_hook = lambda: mod._hook
    mod.set_axon_ntff_profile_hook = lambda h: setattr(mod, "_hook", h)
    sys.modules["antenv.axon_hooks"] = mod
    import antenv

    antenv.axon_hooks = mod


def kernel(images, labels, weight):
    global LAST_EXEC_TIME_NS, LAST_TRACE
    from concourse.bass_utils import run_bass_kernel_spmd

    if "nc" not in _CACHE:
        _CACHE["nc"] = _build()
    nc = _CACHE["nc"]

    in_maps = _prep_in_maps(images, labels, weight)
    trace = bool(int(os.environ.get("KERNEL_TRACE", "0")))
    if trace:
        _install_ntff_hook()
    res = run_bass_kernel_spmd(nc, in_maps, core_ids=list(range(NCORES)), trace=trace)
    LAST_EXEC_TIME_NS = res.exec_time_ns
    LAST_TRACE = res
    out = np.asarray(res.results[0]["out"], dtype=np.float32).reshape(())
    return out
